# revision 14
# baseline (speedup 1.0000x reference)
"""MAGATFN forward pass on 8 Trainium2 NeuronCores (Bass/Tile, SPMD).

Sharding: every core owns a 256-node slice of the node axis (with a 1-node
halo on each side for the node-axis conv + interpolation), computes the
temporal-conv frontend + feature projection for its slice, exchanges k/v via
one AllGather per batch, then runs full graph attention for all 8 heads
restricted to its query slice (adj is pre-transposed + exponentiated on the
host so the learnable-adjacency bias becomes a bf16 elementwise multiply),
and finishes with the multi-scale fusion + prediction backend on its slice.
"""
import sys, os
import numpy as np

for _p in ("/opt/trn_rl_repo", os.path.expanduser("~/.axon_site/_ro/trn_rl_repo")):
    if os.path.isdir(_p) and _p not in sys.path:
        sys.path.insert(0, _p)
        break

import concourse.bass as bass
import concourse.mybir as mybir
import concourse.tile as tile
from concourse.bass_utils import run_bass_kernel_spmd

B, T, N, H, HOR, C = 2, 64, 2048, 128, 24, 32
HEADS, hd = 8, 16
EPS = 1e-5
NCORES = 8
NSH = N // NCORES            # 256
EXT = NSH + 2                # 258 (1-node halo each side)
CT = C * T                   # 2048
NMT = N // 128               # 16 m-tiles over key/value nodes
GRP = 2                      # m-tiles per PSUM group / exp call
CCK = H * NSH                # 32768 floats of k^T per batch
CCV = NSH * H                # 32768 floats of v per batch
CCSZ = CCK + CCV

F32 = mybir.dt.float32
BF16 = mybir.dt.bfloat16
AF = mybir.ActivationFunctionType
ALU = mybir.AluOpType

DMA_ENGINE = "sync"  # switched to "gpsimd" if HWDGE sync-wait limits bite
MAXW = 1             # walrus here rejects >1 semaphore wait per instruction


def split_sync_waits(nc, max_waits=MAXW):
    """Hoist excess semaphore waits onto same-engine NoOps (the walrus build
    in this toolchain rejects instructions with more than ~2 sync waits)."""
    counter = 0
    for fn in nc.m.functions:
        for bb in fn.blocks:
            lst = bb.instructions
            i = 0
            while i < len(lst):
                ins = lst[i]
                si = ins.sync_info
                if si and si.on_wait and len(si.on_wait) > max_waits:
                    waits = list(si.on_wait)
                    excess, keep = waits[:-max_waits], waits[-max_waits:]
                    pos = i
                    for j in range(0, len(excess), max_waits):
                        counter += 1
                        nop = mybir.InstNoOp(
                            name=f"I-wsplit-{counter}",
                            engine=ins.engine,
                            sync_info=mybir.SyncInfo(
                                on_wait=excess[j:j + max_waits], on_update=[]),
                        )
                        lst.insert(pos, nop)
                        pos += 1
                        i += 1
                    si.on_wait = keep
                i += 1
    return counter


def _bf16(a):
    return a.astype(mybir.dt.np(BF16))


def host_prep(inputs):
    """Fold BN/LN/softmax scales into weights; build conv-as-matmul operator."""
    p = {}
    s = np.float32(1.0 / np.sqrt(np.float32(1.0) + np.float32(EPS)))
    tc_w = np.asarray(inputs["tc_w"], np.float32)
    bns = np.asarray(inputs["tc_bn_g"], np.float32) * s
    M = np.zeros((T, CT), np.float32)
    for k in range(3):
        # M[t-1+k, c*T + t] = tc_w[c,0,k] * bns[c]
        tps = np.arange(T) - 1 + k
        valid = (tps >= 0) & (tps < T)
        for c in range(C):
            M[tps[valid], c * T + np.arange(T)[valid]] = tc_w[c, 0, k] * bns[c]
    p["convM"] = M
    p["conv_bias"] = np.repeat(
        np.asarray(inputs["tc_b"], np.float32) * bns + np.asarray(inputs["tc_bn_b"], np.float32), T
    ).astype(np.float32).reshape(16, 128).T.copy()          # [128, 16] col i = chunk i
    C_H = (np.eye(H, dtype=np.float32) - np.float32(1.0 / H))
    p["Wf_l"] = (C_H @ np.asarray(inputs["fp_w"], np.float32)).T.copy()      # [CT, H]
    p["bf_row"] = (C_H @ np.asarray(inputs["fp_b"], np.float32)).reshape(1, H)
    p["fp_gb"] = np.stack([np.asarray(inputs["fp_ln_g"], np.float32),
                           np.asarray(inputs["fp_ln_b"], np.float32)], 1)     # [H, 2]
    sc = np.float32(1.0 / np.sqrt(hd))
    qw = np.asarray(inputs["q_w"], np.float32) * sc
    qb = np.asarray(inputs["q_b"], np.float32) * sc
    QA = np.zeros((H, H), np.float32); QAb = np.zeros((H, 1), np.float32)
    QB = np.zeros((H, H), np.float32); QBb = np.zeros((H, 1), np.float32)
    for hp in range(4):
        QA[32 * hp:32 * hp + 16] = qw[16 * hp:16 * hp + 16]
        QAb[32 * hp:32 * hp + 16, 0] = qb[16 * hp:16 * hp + 16]
        QB[32 * hp:32 * hp + 16] = qw[16 * (hp + 4):16 * (hp + 4) + 16]
        QBb[32 * hp:32 * hp + 16, 0] = qb[16 * (hp + 4):16 * (hp + 4) + 16]
    p["qwA_l"] = QA.T.copy(); p["qbA"] = QAb
    p["qwB_l"] = QB.T.copy(); p["qbB"] = QBb
    p["kw_l"] = np.asarray(inputs["k_w"], np.float32).T.copy()
    p["kb"] = np.asarray(inputs["k_b"], np.float32).reshape(H, 1)
    p["vw_r"] = np.asarray(inputs["v_w"], np.float32).T.copy()
    p["vb_row"] = np.asarray(inputs["v_b"], np.float32).reshape(1, H)
    p["expadjT"] = np.exp(np.asarray(inputs["adj"], np.float32)[0].transpose(0, 2, 1))  # [h, m, n]
    p["ow_l"] = np.asarray(inputs["o_w"], np.float32).T.copy()
    p["ob_row"] = np.asarray(inputs["o_b"], np.float32).reshape(1, H)
    fw = np.asarray(inputs["fusion_weight"], np.float64)
    al = np.exp(fw - fw.max()); al = (al / al.sum()).astype(np.float32)
    g0 = np.asarray(inputs["s0_bn_g"], np.float32) * s
    p["W0_l"] = (al[0] * (g0[:, None] * np.asarray(inputs["s0_w"], np.float32)[:, :, 0])).T.copy()
    p["b0"] = (al[0] * (np.asarray(inputs["s0_b"], np.float32) * g0
                        + np.asarray(inputs["s0_bn_b"], np.float32))).reshape(H, 1)
    g1 = np.asarray(inputs["s1_bn_g"], np.float32) * s
    s1w = np.asarray(inputs["s1_w"], np.float32)
    p["W1a_l"] = (g1[:, None] * s1w[:, :, 0]).T.copy()
    p["W1b_l"] = (g1[:, None] * s1w[:, :, 1]).T.copy()
    p["b1"] = (np.asarray(inputs["s1_b"], np.float32) * g1
               + np.asarray(inputs["s1_bn_b"], np.float32)).reshape(H, 1)
    wvec = ((np.arange(N) + 0.5) / N).astype(np.float32)
    p["Arow"] = al[1] * (1.0 - wvec)
    p["Brow"] = al[1] * wvec
    p["Wfu_l"] = (C_H @ np.asarray(inputs["fu_w"], np.float32)).T.copy()
    p["bfu_row"] = (C_H @ np.asarray(inputs["fu_b"], np.float32)).reshape(1, H)
    p["fu_gb"] = np.stack([np.asarray(inputs["fu_ln_g"], np.float32),
                           np.asarray(inputs["fu_ln_b"], np.float32)], 1)
    p["Wpr_l"] = (C_H @ np.asarray(inputs["pr1_w"], np.float32)).T.copy()
    p["bpr_row"] = (C_H @ np.asarray(inputs["pr1_b"], np.float32)).reshape(1, H)
    p["pr_gb"] = np.stack([np.asarray(inputs["pr_ln_g"], np.float32),
                           np.asarray(inputs["pr_ln_b"], np.float32)], 1)
    p["pr2_l"] = np.asarray(inputs["pr2_w"], np.float32).T.copy()
    p["pr2b_row"] = np.asarray(inputs["pr2_b"], np.float32).reshape(1, HOR)
    p["rg_l"] = np.asarray(inputs["rg_w"], np.float32).T.copy()
    p["rgb_row"] = np.asarray(inputs["rg_b"], np.float32).reshape(1, HOR)
    repl = np.zeros((HEADS, H), np.float32)
    for h in range(HEADS):
        repl[h, 16 * h:16 * h + 16] = 1.0
    p["repl"] = repl
    return p


def per_core_inputs(inputs, p, core):
    s0 = core * NSH
    lo, hi = max(0, s0 - 1), min(N, s0 + NSH + 1)
    a, b = lo - (s0 - 1), hi - (s0 - 1)
    x = np.asarray(inputs["x"], np.float32)
    xe = np.zeros((B, T, EXT), np.float32)
    xe[:, :, a:b] = x[:, :, lo:hi]
    # ghost cols of expadj are 1.0 so softmax denominators stay positive there
    ea = np.ones((HEADS, N, EXT), np.float32)
    ea[:, :, a:b] = p["expadjT"][:, :, lo:hi]
    mask = np.zeros((1, EXT), np.float32)
    mask[0, a:b] = 1.0
    d = {
        "xe": xe,
        "xl24": np.broadcast_to(x[:, T - 1, s0:s0 + NSH][:, None, :], (B, HOR, NSH)).copy(),
        "expadj": _bf16(ea),
        "maskf": np.broadcast_to(mask, (128, EXT)).copy(),
        "Afull": np.broadcast_to(p["Arow"][s0:s0 + NSH][None, :], (128, NSH)).copy(),
        "Bfull": np.broadcast_to(p["Brow"][s0:s0 + NSH][None, :], (128, NSH)).copy(),
        "ones_row": np.ones((1, EXT), np.float32),
        "ones_col": np.ones((128, 1), np.float32),
    }
    for k in ("convM", "conv_bias", "Wf_l", "bf_row", "fp_gb", "qwA_l", "qbA", "qwB_l",
              "qbB", "kw_l", "kb", "vw_r", "vb_row", "ow_l", "ob_row", "W0_l", "b0",
              "W1a_l", "W1b_l", "b1", "Wfu_l", "bfu_row", "fu_gb", "Wpr_l", "bpr_row",
              "pr_gb", "pr2_l", "pr2b_row", "rg_l", "rgb_row", "repl"):
        d[k] = p[k]
    return d


def build_nc():
    nc = bass.Bass()
    dma = getattr(nc, DMA_ENGINE).dma_start

    def inp(name, shape, dt=F32):
        return nc.dram_tensor(name, shape, dt, kind="ExternalInput")

    xe_d = inp("xe", [B, T, EXT])
    xl24_d = inp("xl24", [B, HOR, NSH])
    ea_d = inp("expadj", [HEADS, N, EXT], BF16)
    mask_d = inp("maskf", [128, EXT])
    A_d = inp("Afull", [128, NSH]); Bm_d = inp("Bfull", [128, NSH])
    ones_row_d = inp("ones_row", [1, EXT]); ones_col_d = inp("ones_col", [128, 1])
    convM_d = inp("convM", [T, CT]); conv_bias_d = inp("conv_bias", [128, 16])
    Wf_d = inp("Wf_l", [CT, H]); bf_row_d = inp("bf_row", [1, H]); fp_gb_d = inp("fp_gb", [H, 2])
    qwA_d = inp("qwA_l", [H, H]); qbA_d = inp("qbA", [H, 1])
    qwB_d = inp("qwB_l", [H, H]); qbB_d = inp("qbB", [H, 1])
    kw_d = inp("kw_l", [H, H]); kb_d = inp("kb", [H, 1])
    vw_d = inp("vw_r", [H, H]); vb_row_d = inp("vb_row", [1, H])
    ow_d = inp("ow_l", [H, H]); ob_row_d = inp("ob_row", [1, H])
    W0_d = inp("W0_l", [H, H]); b0_d = inp("b0", [H, 1])
    W1a_d = inp("W1a_l", [H, H]); W1b_d = inp("W1b_l", [H, H]); b1_d = inp("b1", [H, 1])
    Wfu_d = inp("Wfu_l", [H, H]); bfu_row_d = inp("bfu_row", [1, H]); fu_gb_d = inp("fu_gb", [H, 2])
    Wpr_d = inp("Wpr_l", [H, H]); bpr_row_d = inp("bpr_row", [1, H]); pr_gb_d = inp("pr_gb", [H, 2])
    pr2_d = inp("pr2_l", [H, HOR]); pr2b_row_d = inp("pr2b_row", [1, HOR])
    rg_d = inp("rg_l", [H, HOR]); rgb_row_d = inp("rgb_row", [1, HOR])
    repl_d = inp("repl", [HEADS, H])

    out_d = nc.dram_tensor("out", [B, HOR, NSH], F32, kind="ExternalOutput")

    cc_in = [nc.dram_tensor(f"cc_in{b}", [CCSZ], F32) for b in range(B)]
    cc_out = [nc.dram_tensor(f"cc_out{b}", [NCORES, CCSZ], F32, addr_space="Shared")
              for b in range(B)]

    with tile.TileContext(nc) as tc:
        with (
            tc.tile_pool(name="singles", bufs=1) as singles,
            tc.tile_pool(name="fpool", bufs=3) as fpool,
            tc.tile_pool(name="featsp", bufs=2) as featsp,
            tc.tile_pool(name="qp", bufs=2) as qp,
            tc.tile_pool(name="kvp", bufs=2) as kvp,
            tc.tile_pool(name="khp", bufs=1) as khp,
            tc.tile_pool(name="vap", bufs=16) as vap,
            tc.tile_pool(name="eap", bufs=3) as eap,
            tc.tile_pool(name="eqp", bufs=3) as eqp,
            tc.tile_pool(name="ep", bufs=3) as ep,
            tc.tile_pool(name="bk", bufs=2) as bk,
            tc.tile_pool(name="spool", bufs=2, space="PSUM") as spool,
            tc.tile_pool(name="avpool", bufs=2, space="PSUM") as avpool,
            tc.tile_pool(name="accp", bufs=1, space="PSUM") as accp,
            tc.tile_pool(name="tmpp", bufs=1, space="PSUM") as tmpp,
        ):
            # ------- load constants/weights into SBUF -------
            def lw(d, shape, dt=F32):
                t = singles.tile(shape, dt, tag=f"w_{d.name}")
                dma(t[:], d[:])
                return t

            convM_s = lw(convM_d, [T, CT])
            Wf_s = singles.tile([128, CT], F32)
            dma(Wf_s[:].rearrange("p (i j) -> p i j", i=16),
                Wf_d.rearrange("(i p) j -> p i j", p=128))
            conv_bias_s = lw(conv_bias_d, [128, 16])
            bf_row_s = lw(bf_row_d, [1, H]); fp_gb_s = lw(fp_gb_d, [H, 2])
            qwA_s = lw(qwA_d, [H, H]); qbA_s = lw(qbA_d, [H, 1])
            qwB_s = lw(qwB_d, [H, H]); qbB_s = lw(qbB_d, [H, 1])
            kw_s = lw(kw_d, [H, H]); kb_s = lw(kb_d, [H, 1])
            vw_s = lw(vw_d, [H, H]); vb_row_s = lw(vb_row_d, [1, H])
            ow_s = lw(ow_d, [H, H]); ob_row_s = lw(ob_row_d, [1, H])
            W0_s = lw(W0_d, [H, H]); b0_s = lw(b0_d, [H, 1])
            W1a_s = lw(W1a_d, [H, H]); W1b_s = lw(W1b_d, [H, H]); b1_s = lw(b1_d, [H, 1])
            Wfu_s = lw(Wfu_d, [H, H]); bfu_row_s = lw(bfu_row_d, [1, H]); fu_gb_s = lw(fu_gb_d, [H, 2])
            Wpr_s = lw(Wpr_d, [H, H]); bpr_row_s = lw(bpr_row_d, [1, H]); pr_gb_s = lw(pr_gb_d, [H, 2])
            pr2_s = lw(pr2_d, [H, HOR]); pr2b_row_s = lw(pr2b_row_d, [1, HOR])
            rg_s = lw(rg_d, [H, HOR]); rgb_row_s = lw(rgb_row_d, [1, HOR])
            repl_s = lw(repl_d, [HEADS, H])
            mask_s = lw(mask_d, [128, EXT])
            A_s = lw(A_d, [128, NSH]); Bm_s = lw(Bm_d, [128, NSH])
            ones_row_s = lw(ones_row_d, [1, EXT]); ones_col_s = lw(ones_col_d, [128, 1])

            ORow = ones_row_s  # [1, EXT] — K=1 bias matmuls use slices of this
            eps_s = singles.tile([1, 1], F32)
            nc.gpsimd.memset(eps_s[:], float(EPS))

            def bias_mm(psum_ap, row_s, n, start=False, stop=False):
                """psum[p, 0:n] += row_s[0, p] (rank-1 bias via K=1 matmul)."""
                nc.tensor.matmul(psum_ap, row_s[:], ORow[0:1, 0:n], start=start, stop=stop)

            def ln_norm(zc_psum, n, gb_s, out_tile):
                """out = relu(g * (zc * rsqrt(mean(zc^2)+eps)) + b), mean over 128 partitions."""
                sq = bk.tile([128, EXT], F32, tag="sq")
                nc.scalar.activation(sq[:, 0:n], zc_psum, AF.Square)
                S = tmpp.tile([128, 512], F32, tag="tmp")
                nc.tensor.matmul(S[0:1, 0:n], ones_col_s[:], sq[:, 0:n], start=True, stop=True)
                t1 = bk.tile([1, EXT], F32, tag="lnt1")
                nc.scalar.activation(t1[0:1, 0:n], S[0:1, 0:n], AF.Ln,
                                     bias=eps_s[0:1, 0:1], scale=float(1.0 / H))
                inv = bk.tile([1, EXT], F32, tag="lninv")
                nc.scalar.activation(inv[0:1, 0:n], t1[0:1, 0:n], AF.Exp, scale=-0.5)
                I = tmpp.tile([128, 512], F32, tag="tmp")
                nc.tensor.matmul(I[:, 0:n], ones_row_s[0:1, 0:128], inv[0:1, 0:n],
                                 start=True, stop=True)
                invb = bk.tile([128, EXT], F32, tag="lninvb")
                nc.scalar.copy(invb[:, 0:n], I[:, 0:n])
                zn = bk.tile([128, EXT], F32, tag="lnzn")
                nc.vector.tensor_tensor(zn[:, 0:n], zc_psum, invb[:, 0:n], op=ALU.mult)
                nc.scalar.activation(out_tile, zn[:, 0:n], AF.Relu,
                                     bias=gb_s[:, 1:2], scale=gb_s[:, 0:1])

            featsT = [None, None]
            qflat = [None, None]
            for b in range(B):
                # ---------------- frontend ----------------
                x_s = bk.tile([T, EXT], F32, tag="xsb")
                dma(x_s[:], xe_d[b])
                fe = accp.tile([128, 512], F32, tag="acc")
                for i in range(16):
                    cp = tmpp.tile([128, 512], F32, tag="tmp")
                    nc.tensor.matmul(cp[:, 0:EXT], convM_s[:, 128 * i:128 * (i + 1)],
                                     x_s[:], start=True, stop=True)
                    f_s = fpool.tile([128, EXT], F32)
                    if i % 2 == 0:
                        nc.scalar.activation(f_s[:], cp[:, 0:EXT], AF.Relu,
                                             bias=conv_bias_s[:, i:i + 1])
                    else:
                        nc.vector.tensor_scalar(f_s[:], cp[:, 0:EXT],
                                                conv_bias_s[:, i:i + 1], 0.0,
                                                op0=ALU.add, op1=ALU.max)
                    nc.tensor.matmul(fe[:, 0:EXT], Wf_s[:, 128 * i:128 * (i + 1)],
                                     f_s[:], start=(i == 0), stop=False)
                bias_mm(fe[:, 0:EXT], bf_row_s, EXT, stop=True)
                ft = featsp.tile([128, EXT], F32)
                ln_norm(fe[:, 0:EXT], EXT, fp_gb_s, ft[:])
                featsT[b] = ft

                # ---------------- q / k / v + AllGather ----------------
                qf = qp.tile([16, HEADS * EXT], F32, tag="qflat")
                for half, (w_s, b_s) in enumerate(((qwA_s, qbA_s), (qwB_s, qbB_s))):
                    qps = accp.tile([128, 512], F32, tag="acc")
                    nc.tensor.matmul(qps[:, 0:EXT], w_s[:], ft[:], start=True, stop=True)
                    q_s = qp.tile([128, EXT], F32, tag="qpad")
                    nc.vector.tensor_scalar(q_s[:], qps[:, 0:EXT], b_s[:], None, op0=ALU.add)
                    # repack padded heads [32hp:32hp+16, :] -> qflat[:, (4*half+hp)*EXT :]
                    for hp in range(4):
                        hh_ = 4 * half + hp
                        dma(qf[:, hh_ * EXT:(hh_ + 1) * EXT],
                            q_s[32 * hp:32 * hp + 16, :])
                qflat[b] = qf

                kps = accp.tile([128, 512], F32, tag="acc")
                nc.tensor.matmul(kps[:, 0:NSH], kw_s[:], ft[:, 1:1 + NSH], start=True, stop=True)
                k_s = kvp.tile([128, NSH], F32, tag="ksb")
                nc.vector.tensor_scalar(k_s[:], kps[:, 0:NSH], kb_s[:], None, op0=ALU.add)
                dma(cc_in[b][0:CCK].rearrange("(p f) -> p f", p=H), k_s[:])
                for mt in range(2):
                    vps = accp.tile([128, 512], F32, tag="acc")
                    nc.tensor.matmul(vps[:, 0:H], ft[:, 1 + 128 * mt:1 + 128 * (mt + 1)],
                                     vw_s[:], start=True, stop=False)
                    nc.tensor.matmul(vps[:, 0:H], ones_row_s[0:1, 0:128], vb_row_s[:],
                                     start=False, stop=True)
                    v_s = kvp.tile([128, H], F32, tag="vsb")
                    nc.vector.tensor_copy(v_s[:], vps[:, 0:H])
                    dma(cc_in[b][CCK + 128 * H * mt:CCK + 128 * H * (mt + 1)]
                        .rearrange("(p f) -> p f", p=128), v_s[:])
                nc.gpsimd.collective_compute(
                    "AllGather", ALU.bypass,
                    replica_groups=[list(range(NCORES))],
                    ins=[cc_in[b][:]], outs=[cc_out[b][:]],
                )

            goT = [None, None]
            for b in range(B):
                # ---------------- gather k/v from collective ----------------
                kh = khp.tile([16, HEADS * N], F32, tag="khead")
                co = cc_out[b]
                co_k = co[:, 0:CCK].rearrange("r (d j) -> d r j", d=H)  # [H, 8, NSH]
                for h in range(HEADS):
                    # kh[d, h*N + r*NSH + j] = co[r, (16h+d)*NSH + j]
                    dma(kh[:, h * N:(h + 1) * N].rearrange("d (r j) -> d r j", r=NCORES),
                        co_k[16 * h:16 * h + 16])
                va = []
                for t in range(NMT):
                    vt = vap.tile([128, HEADS * 33], F32, tag="vaug")
                    r, half = t // 2, t % 2
                    vsrc = co[r, CCK + 128 * H * half:CCK + 128 * H * (half + 1)]
                    vt3 = vt[:].rearrange("p (h e) -> p h e", h=HEADS)
                    dma(vt3[:, :, 0:16], vsrc.rearrange("(p h e) -> p h e", p=128, h=HEADS))
                    nc.gpsimd.memset(vt3[:, :, 16:32], 0.0)
                    nc.gpsimd.memset(vt3[:, :, 32:33], 1.0)
                    va.append(vt)

                # ---------------- attention per head ----------------
                drows = bk.tile([HEADS, EXT], F32, tag="drows")
                goraw = bk.tile([128, EXT], F32, tag="goraw")
                for h in range(HEADS):
                    avp = avpool.tile([33, EXT], F32, tag="av")
                    for g in range(NMT // GRP):
                        sps = spool.tile([128, 1024], F32, tag="sc")
                        for i in range(GRP):
                            mt = GRP * g + i
                            nc.tensor.matmul(
                                sps[:, 512 * i:512 * i + EXT],
                                kh[:, h * N + 128 * mt:h * N + 128 * (mt + 1)],
                                qflat[b][:, h * EXT:(h + 1) * EXT],
                                start=True, stop=True)
                        eq = eqp.tile([128, GRP * EXT], BF16, tag="eq")
                        nc.scalar.activation(
                            eq[:].rearrange("p (i n) -> p i n", i=GRP),
                            sps[:].rearrange("p (i n) -> p i n", i=GRP)[:, :, 0:EXT],
                            AF.Exp)
                        ea_t = eap.tile([128, GRP * EXT], BF16, tag="ea")
                        dma(ea_t[:].rearrange("p (i n) -> p i n", i=GRP),
                            ea_d.rearrange("h (g2 i p) n -> h g2 i p n",
                                           i=GRP, p=128)[h, g].rearrange("i p n -> p i n"))
                        et = ep.tile([128, GRP * EXT], F32, tag="et")
                        nc.vector.tensor_tensor(et[:], eq[:], ea_t[:], op=ALU.mult)
                        for i in range(GRP):
                            mt = GRP * g + i
                            nc.tensor.matmul(
                                avp[:], va[mt][:, 33 * h:33 * h + 33],
                                et[:, EXT * i:EXT * (i + 1)],
                                start=(mt == 0), stop=(mt == NMT - 1))
                    # engine accesses need 32-aligned partition bases; rows
                    # 16..31 of avp are zeros (zero lhsT columns), D is at 32.
                    stag = bk.tile([32, EXT], F32, tag="avstag")
                    nc.vector.tensor_copy(stag[:], avp[0:32, :])
                    d1 = bk.tile([1, EXT], F32, tag="d1")
                    nc.vector.reciprocal(d1[:], avp[32:33, :])
                    dma(goraw[16 * h:16 * h + 16, :], stag[0:16, :])
                    dma(drows[h:h + 1, :], d1[:])
                dps = tmpp.tile([128, 512], F32, tag="tmp")
                nc.tensor.matmul(dps[:, 0:EXT], repl_s[:], drows[:], start=True, stop=True)
                dinv = bk.tile([128, EXT], F32, tag="dinv")
                nc.scalar.copy(dinv[:], dps[:, 0:EXT])
                go = bk.tile([128, EXT], F32, tag="goT")
                nc.vector.tensor_tensor(go[:], goraw[:], dinv[:], op=ALU.mult)
                goT[b] = go

            for b in range(B):
                # ---------------- backend ----------------
                gps = accp.tile([128, 512], F32, tag="acc")
                nc.tensor.matmul(gps[:, 0:EXT], ow_s[:], goT[b][:], start=True, stop=False)
                bias_mm(gps[:, 0:EXT], ob_row_s, EXT, stop=True)
                gf = bk.tile([128, EXT], F32, tag="gf")
                nc.vector.tensor_tensor(gf[:], gps[:, 0:EXT], mask_s[:], op=ALU.mult)

                f0ps = accp.tile([128, 512], F32, tag="acc")
                nc.tensor.matmul(f0ps[:, 0:EXT], W0_s[:], gf[:], start=True, stop=True)
                f0 = bk.tile([128, EXT], F32, tag="f0")
                nc.scalar.activation(f0[:], f0ps[:, 0:EXT], AF.Relu, bias=b0_s[:])

                o1ps = accp.tile([128, 512], F32, tag="acc")
                nc.tensor.matmul(o1ps[:, 0:EXT - 1], W1a_s[:], gf[:, 0:EXT - 1],
                                 start=True, stop=False)
                nc.tensor.matmul(o1ps[:, 0:EXT - 1], W1b_s[:], gf[:, 1:EXT],
                                 start=False, stop=True)
                O1 = bk.tile([128, EXT - 1], F32, tag="O1")
                nc.scalar.activation(O1[:], o1ps[:, 0:EXT - 1], AF.Relu, bias=b1_s[:])

                t1 = bk.tile([128, NSH], F32, tag="fu1")
                nc.vector.tensor_tensor(t1[:], O1[:, 0:NSH], A_s[:], op=ALU.mult)
                t2 = bk.tile([128, NSH], F32, tag="fu2")
                nc.vector.tensor_tensor(t2[:], O1[:, 1:1 + NSH], Bm_s[:], op=ALU.mult)
                nc.vector.tensor_tensor(t1[:], t1[:], t2[:], op=ALU.add)
                fused = bk.tile([128, NSH], F32, tag="fused")
                nc.vector.tensor_tensor(fused[:], t1[:], f0[:, 1:1 + NSH], op=ALU.add)

                zps = accp.tile([128, 512], F32, tag="acc")
                nc.tensor.matmul(zps[:, 0:NSH], Wfu_s[:], fused[:], start=True, stop=False)
                bias_mm(zps[:, 0:NSH], bfu_row_s, NSH, stop=True)
                ff = bk.tile([128, NSH], F32, tag="ff")
                ln_norm(zps[:, 0:NSH], NSH, fu_gb_s, ff[:])

                z2ps = accp.tile([128, 512], F32, tag="acc")
                nc.tensor.matmul(z2ps[:, 0:NSH], Wpr_s[:], ff[:], start=True, stop=False)
                bias_mm(z2ps[:, 0:NSH], bpr_row_s, NSH, stop=True)
                hh = bk.tile([128, NSH], F32, tag="hh")
                ln_norm(z2ps[:, 0:NSH], NSH, pr_gb_s, hh[:])

                pps = accp.tile([128, 512], F32, tag="acc")
                nc.tensor.matmul(pps[0:HOR, 0:NSH], pr2_s[:], hh[:], start=True, stop=False)
                nc.tensor.matmul(pps[0:HOR, 0:NSH], pr2b_row_s[:], ORow[0:1, 0:NSH],
                                 start=False, stop=True)
                preds = bk.tile([HOR, NSH], F32, tag="preds")
                nc.vector.tensor_copy(preds[:], pps[0:HOR, 0:NSH])

                gaps = accp.tile([128, 512], F32, tag="acc")
                nc.tensor.matmul(gaps[0:HOR, 0:NSH], rg_s[:], ff[:], start=True, stop=False)
                nc.tensor.matmul(gaps[0:HOR, 0:NSH], rgb_row_s[:], ORow[0:1, 0:NSH],
                                 start=False, stop=True)
                esb = bk.tile([HOR, NSH], F32, tag="esb")
                nc.scalar.activation(esb[:], gaps[0:HOR, 0:NSH], AF.Exp, scale=-1.0)
                nc.vector.tensor_scalar(esb[:], esb[:], 1.0, None, op0=ALU.add)
                gates = bk.tile([HOR, NSH], F32, tag="gates")
                nc.vector.reciprocal(gates[:], esb[:])

                xl = bk.tile([HOR, NSH], F32, tag="xl")
                dma(xl[:], xl24_d[b])
                nc.vector.tensor_tensor(xl[:], xl[:], preds[:], op=ALU.subtract)
                nc.vector.tensor_tensor(xl[:], xl[:], gates[:], op=ALU.mult)
                outt = bk.tile([HOR, NSH], F32, tag="outt")
                nc.vector.tensor_tensor(outt[:], xl[:], preds[:], op=ALU.add)
                dma(out_d[b], outt[:])

    split_sync_waits(nc)
    return nc


_NC_CACHE = {}


def kernel(**inputs):
    p = host_prep(inputs)
    in_maps = [per_core_inputs(inputs, p, c) for c in range(NCORES)]
    if "nc" not in _NC_CACHE:
        _NC_CACHE["nc"] = build_nc()
    res = run_bass_kernel_spmd(_NC_CACHE["nc"], in_maps, list(range(NCORES)))
    out = np.concatenate([res.results[c]["out"] for c in range(NCORES)], axis=2)
    reg = np.float32(1e-4) / np.float32(N)
    return out, reg


if __name__ == "__main__":
    rng = np.random.default_rng(0)
    demo = {"x": rng.standard_normal((B, T, N)).astype(np.float32)}
    print("kernel.py is a library; use test.py")
